# revision 26
# baseline (speedup 1.0000x reference)
"""Trainium2 Bass kernel for LoFTR-style encoder layer (sparse attention + convs).

Sharding: pure data-parallel over batch B=8 -> 8 NeuronCores (one batch
element per core). BN statistics are all-reduced across cores.

V2 restructure vs baseline:
  - host pre-pads + pre-casts feat0 to bf16 in the conv-input layout
    (ip1 f-halves DMA straight in; no on-device casts / big memsets)
  - conv1 f-only tiles (j=0..6) run while the K/V elu chain executes;
    tiles 7..15 interleave with the Q/message loop (attention step jj
    unlocks conv1 tile 7+jj)
  - DVE bn_stats/bn_aggr for BN statistics; conv psum evacuation on the
    scalar (ACT) engine; elu min/eps-add on gpsimd
  - BN1-apply chunked and interleaved with conv2 tiles
  - residual read from the resident bf16 ip1 copy (no f32 reload)
"""

import os
import sys

import numpy as np

for _p in ("/opt/trn_rl_repo", os.path.expanduser("~/.axon_site/_ro/trn_rl_repo")):
    if os.path.isdir(_p) and _p not in sys.path:
        sys.path.insert(0, _p)

import ml_dtypes

import concourse.bass as bass
import concourse.mybir as mybir
import concourse.tile as tile
from concourse import bacc
from concourse.bass_utils import run_bass_kernel_spmd

F32 = mybir.dt.float32
BF16 = mybir.dt.bfloat16
AF = mybir.ActivationFunctionType
ALU = mybir.AluOpType

NCORES = 8
H = W = 80
HW = H * W          # 6400
D = 256
NI = 3200           # inside positions (image rows 0..39)
NHEAD = 8
PW = W + 2          # 82 padded width
ATTN_EPS = 1e-6
BN_EPS = 1e-5
BN_N = float(NCORES * HW)

# conv row-tiling: 5 output rows per psum tile -> N = 5*82 = 410 <= 512
RT = 5
NRT = H // RT       # 16
NT = RT * PW        # 410

LAST_EXEC_NS = None
LAST_MEAN_EXEC_NS = None

_cache = {}


def _bd(ap3):
    return ap3.rearrange("p a b -> p (a b)")


def _r3(ap2, a):
    return ap2.rearrange("p (a b) -> p a b", a=a)


def build_nc():
    nc = bacc.Bacc(
        "TRN2", target_bir_lowering=False, debug=False, num_devices=NCORES
    )

    ftpad_d = nc.dram_tensor("ftpad", [128, 2, 84 * PW], BF16, kind="ExternalInput")
    ftin_d = nc.dram_tensor("ftin", [128, 2, NI], BF16, kind="ExternalInput")
    wqt_d = nc.dram_tensor("wqt", [128, 2, D], BF16, kind="ExternalInput")
    wkt_d = nc.dram_tensor("wkt", [128, 2, D], BF16, kind="ExternalInput")
    wvt_d = nc.dram_tensor("wvt", [128, 2, D], BF16, kind="ExternalInput")
    c1w_d = nc.dram_tensor("c1w", [128, 36, D], BF16, kind="ExternalInput")
    c2w_d = nc.dram_tensor("c2w", [128, 18, D], BF16, kind="ExternalInput")
    bn1g_d = nc.dram_tensor("bn1g", [D, 1], F32, kind="ExternalInput")
    bn1b_d = nc.dram_tensor("bn1b", [D, 1], F32, kind="ExternalInput")
    bn2g_d = nc.dram_tensor("bn2g", [D, 1], F32, kind="ExternalInput")
    bn2b_d = nc.dram_tensor("bn2b", [D, 1], F32, kind="ExternalInput")
    mblk_d = nc.dram_tensor("mblk", [8, 256], BF16, kind="ExternalInput")
    out_d = nc.dram_tensor("out_t", [D, HW], BF16, kind="ExternalOutput")

    groups = [list(range(NCORES))]

    with tile.TileContext(nc) as tc:
        with (
            tc.tile_pool(name="pers", bufs=1) as pers,
            tc.tile_pool(name="bigp", bufs=2) as bigp,
            tc.tile_pool(name="b2p", bufs=4) as b2p,
            tc.tile_pool(name="qtp", bufs=4) as qtp,
            tc.tile_pool(name="scr", bufs=5) as scr,
            tc.tile_pool(name="small", bufs=1) as small,
            tc.tile_pool(name="fin", bufs=3) as fin,
            tc.tile_pool(name="finO", bufs=4) as finO,
            tc.tile_pool(name="psA", bufs=3, space="PSUM") as psA,
            tc.tile_pool(name="psS", bufs=1, space="PSUM") as psS,
            tc.tile_pool(name="psC", bufs=4, space="PSUM") as psC,
            tc.tile_pool(name="dram", bufs=1, space="DRAM") as dramp,
        ):
            # ---------------- weights / consts / feat loads ----------------
            wqt = pers.tile([128, 2, D], BF16, tag="wqt", name="wqt")
            wkt = pers.tile([128, 2, D], BF16, tag="wkt", name="wkt")
            wvt = pers.tile([128, 2, D], BF16, tag="wvt", name="wvt")
            c1w = pers.tile([128, 36, D], BF16, tag="c1w", name="c1w")
            c2w = pers.tile([128, 18, D], BF16, tag="c2w", name="c2w")
            ftin = pers.tile([128, 2, NI], BF16, tag="ftin", name="ftin")
            maskblk = pers.tile([8, 256], BF16, tag="maskblk", name="maskblk")
            # padded conv1 input [512ch -> 4 chunks][84 rows, 82 cols]
            # tile row r+2 == image row r; guard rows/cols around.
            ip1 = [
                pers.tile([128, 84, PW], BF16, tag=f"ip1_{c}", name=f"ip1_{c}")
                for c in range(4)
            ]
            # dummy collective: warms the CC path / absorbs start skew
            # while phase-A compute proceeds
            dums = small.tile([8, 1], F32, tag="dums", name="dums")
            nc.gpsimd.memset(dums[:, :], 0.0)
            dumi = dramp.tile([8, 1], F32, tag="dumi", name="dumi")
            dumo = dramp.tile([8, 1], F32, tag="dumo", name="dumo")
            nc.sync.dma_start(dumi[:, :], dums[:, :])
            nc.gpsimd.collective_compute(
                "AllReduce", ALU.add, replica_groups=groups,
                ins=[dumi[:, :].opt()], outs=[dumo[:, :].opt()],
            )
            nc.sync.dma_start(wkt[:, :, :], wkt_d[:, :, :])
            for fc in range(4):
                nc.sync.dma_start(
                    ftin[:, :, 800 * fc : 800 * (fc + 1)],
                    ftin_d[:, :, 800 * fc : 800 * (fc + 1)],
                )
            nc.sync.dma_start(wvt[:, :, :], wvt_d[:, :, :])
            for m in range(2):
                nc.sync.dma_start(ip1[m][:, 0:42, :], _r3(ftpad_d[:, m, :], 84)[:, 0:42, :])
            nc.sync.dma_start(c1w[:, 0:18, :], c1w_d[:, 0:18, :])
            for m in range(2):
                nc.sync.dma_start(ip1[m][:, 42:84, :], _r3(ftpad_d[:, m, :], 84)[:, 42:84, :])
            nc.sync.dma_start(c1w[:, 18:36, :], c1w_d[:, 18:36, :])
            nc.sync.dma_start(wqt[:, :, :], wqt_d[:, :, :])
            nc.sync.dma_start(maskblk[:, :], mblk_d[:, :])
            nc.sync.dma_start(c2w[:, :, :], c2w_d[:, :, :])

            eps_t = small.tile([128, 1], F32, tag="eps_t", name="eps_t")
            nc.gpsimd.memset(eps_t[:, :], BN_EPS)
            epsA = small.tile([128, 1], F32, tag="epsA", name="epsA")
            nc.gpsimd.memset(epsA[:, :], ATTN_EPS)
            g1 = small.tile([128, 2], F32, tag="g1", name="g1")
            b1 = small.tile([128, 2], F32, tag="b1", name="b1")
            g2 = small.tile([128, 2], F32, tag="g2", name="g2")
            b2 = small.tile([128, 2], F32, tag="b2", name="b2")
            for o in range(2):
                sl = slice(o * 128, (o + 1) * 128)
                nc.sync.dma_start(g1[:, o : o + 1], bn1g_d[sl, :])
                nc.sync.dma_start(b1[:, o : o + 1], bn1b_d[sl, :])
                nc.sync.dma_start(g2[:, o : o + 1], bn2g_d[sl, :])
                nc.sync.dma_start(b2[:, o : o + 1], bn2b_d[sl, :])

            # t-halves: zero region (image rows < 40 and halo)
            for c in (2, 3):
                nc.gpsimd.memset(ip1[c][:, 0:42, :], 0.0)
                nc.gpsimd.memset(ip1[c][:, 82:84, :], 0.0)
                nc.gpsimd.memset(ip1[c][:, 42:82, 0:1], 0.0)
                nc.gpsimd.memset(ip1[c][:, 42:82, 81:82], 0.0)

            # ---------------- conv1 machinery ----------------
            # y1/y2 hold valid cols only ([128, 80, 80])
            y1 = [
                b2p.tile([128, H, W], BF16, tag="b2", name=f"y1_{o}")
                for o in range(2)
            ]
            st1 = [
                small.tile([128, NRT, 6], F32, tag=f"st_{o}", name=f"st1_{o}")
                for o in range(2)
            ]
            ip1f = [_bd(ip1[c][:, :, :]) for c in range(4)]

            def conv1_half(j, o):
                r0 = RT * j
                taps = []
                for c in range(4):
                    for ky in range(3):
                        if c >= 2 and r0 + ky + 4 < 41:
                            continue  # t-channel rows all zero
                        for kx in range(3):
                            taps.append((c, ky, kx))
                ps = psC.tile([128, NT], F32, tag="psC", name="psC")
                for idx, (c, ky, kx) in enumerate(taps):
                    s = (r0 + ky + 1) * PW + kx - 1
                    nc.tensor.matmul(
                        ps[:, :],
                        c1w[:, c * 9 + ky * 3 + kx, o * 128 : (o + 1) * 128],
                        ip1f[c][:, s : s + NT],
                        start=(idx == 0),
                        stop=(idx == len(taps) - 1),
                    )
                val = _r3(ps[:, :], RT)[:, :, 1:81]
                nc.scalar.copy(y1[o][:, r0 : r0 + RT, :], val)
                nc.vector.bn_stats(
                    st1[o][:, j, :],
                    _bd(y1[o][:, :, :])[:, r0 * W : r0 * W + 400],
                )

            def conv1_tile(j):
                conv1_half(j, 0)
                conv1_half(j, 1)

            # ---------------- K / V projections ([s, c] layout) ----------------
            ke = bigp.tile([128, 25, D], BF16, tag="big", name="ke")
            ve = bigp.tile([128, 25, D + 1], BF16, tag="big", name="ve")
            nc.gpsimd.memset(ve[:, :, :], 1.0)  # col 0 of each chunk = ones
            for i in range(25):
                ps = psA.tile([128, D], F32, tag="psA", name="psA")
                for ki in range(2):
                    nc.tensor.matmul(
                        ps[:, :],
                        ftin[:, ki, i * 128 : (i + 1) * 128],
                        wkt[:, ki, :],
                        start=(ki == 0),
                        stop=(ki == 1),
                    )
                # elu(x)+1 = relu(x) + exp(min(x,0))
                sm = scr.tile([128, 400], F32, tag="scr", name="sm")
                se = scr.tile([128, 400], F32, tag="scr", name="se")
                nc.vector.tensor_scalar_min(sm[:, :D], ps[:, :], 0.0)
                nc.scalar.activation(se[:, :D], sm[:, :D], AF.Exp)
                nc.vector.scalar_tensor_tensor(
                    ke[:, i, :], ps[:, :], 0.0, se[:, :D], ALU.max, ALU.add
                )

                ps2 = psC.tile([128, D], F32, tag="psC", name="psC")
                for ki in range(2):
                    nc.tensor.matmul(
                        ps2[:, :],
                        ftin[:, ki, i * 128 : (i + 1) * 128],
                        wvt[:, ki, :],
                        start=(ki == 0),
                        stop=(ki == 1),
                    )
                nc.scalar.copy(ve[:, i, 1:], ps2[:, :])

                if i % 4 == 3:
                    conv1_tile(i // 4)

            # ---------------- KV + Ksum -> block-diag BD ----------------
            bd = [
                pers.tile([128, 264], BF16, tag=f"bd{m}", name=f"bd{m}")
                for m in range(2)
            ]
            for m in range(2):
                psm = psA.tile([128, D + 1], F32, tag="psA", name="psA")
                for i in range(25):
                    nc.tensor.matmul(
                        psm[:, :],
                        ke[:, i, m * 128 : (m + 1) * 128],
                        ve[:, i, :],
                        start=(i == 0),
                        stop=(i == 24),
                    )
                nc.gpsimd.memset(bd[m][:, :], 0.0)
                for hh in range(4):
                    h = m * 4 + hh
                    lh = hh * 32
                    nc.vector.tensor_copy(
                        bd[m][lh : lh + 32, h * 32 : (h + 1) * 32],
                        psm[lh : lh + 32, 1 + h * 32 : 1 + (h + 1) * 32],
                    )
                    nc.vector.tensor_copy(
                        bd[m][lh : lh + 32, 256 + h : 257 + h],
                        psm[lh : lh + 32, 0:1],
                    )

            # ---------- per l-tile: Q^T proj + elu, S, message, scatter ----------
            # conv1 tile 7+jj interleaves after attention step jj
            for jj in range(8):
                qt = [
                    qtp.tile([128, 400], BF16, tag="qteT", name=f"qt{m}")
                    for m in range(2)
                ]
                for m in range(2):
                    ps = psA.tile([128, 400], F32, tag="psA", name="psA")
                    for ki in range(2):
                        nc.tensor.matmul(
                            ps[:, :],
                            wqt[:, ki, m * 128 : (m + 1) * 128],
                            ip1[ki][:, 42 + 5 * jj : 47 + 5 * jj, 1:81],
                            start=(ki == 0),
                            stop=(ki == 1),
                        )
                    sm = scr.tile([128, 400], F32, tag="scr", name="smq")
                    se = scr.tile([128, 400], F32, tag="scr", name="seq")
                    nc.vector.tensor_scalar_min(sm[:, :], ps[:, :], 0.0)
                    nc.scalar.activation(se[:, :], sm[:, :], AF.Exp)
                    nc.vector.scalar_tensor_tensor(
                        qt[m][:, :], ps[:, :], 0.0, se[:, :], ALU.max, ALU.add
                    )

                conv1_half(6 + jj, 0)

                pss = psS.tile([8, 400], F32, tag="psS", name="psS")
                for ki in range(2):
                    nc.tensor.matmul(
                        pss[:, :],
                        bd[ki][:, 256:264],
                        qt[ki][:, :],
                        start=(ki == 0),
                        stop=(ki == 1),
                    )
                sadd = scr.tile([128, 400], F32, tag="scr", name="sadd")
                rs = scr.tile([128, 400], BF16, tag="scr", name="rs")
                nc.vector.tensor_scalar_add(sadd[:8, :], pss[:, :], ATTN_EPS)
                with nc.allow_low_precision(reason="Z recip feeds bf16 conv"):
                    nc.vector.reciprocal(rs[:8, :], sadd[:8, :])

                conv1_half(6 + jj, 1)

                pre = psC.tile([128, 400], F32, tag="psC", name="psC")
                nc.tensor.matmul(pre[:, :], maskblk[:, 0:128], rs[:8, :])
                preb = scr.tile([128, 400], F32, tag="scr", name="preb")
                nc.scalar.copy(preb[:, :], pre[:, :])
                pre2 = psC.tile([128, 400], F32, tag="psC", name="psC")
                nc.tensor.matmul(pre2[:, :], maskblk[:, 128:256], rs[:8, :])
                preb2 = scr.tile([128, 400], F32, tag="scr", name="preb2")
                nc.scalar.copy(preb2[:, :], pre2[:, :])
                prebs = [preb, preb2]

                for m in range(2):
                    psg = psA.tile([128, 400], F32, tag="psA", name="psA")
                    for ki in range(2):
                        nc.tensor.matmul(
                            psg[:, :],
                            bd[ki][:, m * 128 : (m + 1) * 128],
                            qt[ki][:, :],
                            start=(ki == 0),
                            stop=(ki == 1),
                        )
                    psgb = scr.tile([128, 400], F32, tag="scr", name="psgb")
                    nc.scalar.copy(psgb[:, :], psg[:, :])
                    # l-tile jj = image rows 40+5jj..44+5jj -> tile rows 42+5jj..
                    nc.gpsimd.tensor_tensor(
                        ip1[2 + m][:, 42 + 5 * jj : 47 + 5 * jj, 1:81],
                        _r3(psgb[:, :], RT),
                        _r3(prebs[m][:, :], RT),
                        ALU.mult,
                    )

            conv1_tile(14)
            conv1_tile(15)

            # ---------------- BN1 stats allreduce ----------------
            def bn_global(st, tag):
                bnst = small.tile([128, 4], F32, tag=f"bnst{tag}", name=f"bnst{tag}")
                for o in range(2):
                    mv = small.tile([128, 2], F32, tag=f"mv{tag}{o}", name=f"mv{tag}{o}")
                    nc.vector.bn_aggr(mv[:, :], st[o][:, :, :])
                    msq = small.tile([128, 1], F32, tag=f"msq{tag}{o}", name=f"msq{tag}{o}")
                    vps = small.tile([128, 1], F32, tag=f"vps{tag}{o}", name=f"vps{tag}{o}")
                    nc.vector.tensor_tensor(
                        msq[:, :], mv[:, 0:1], mv[:, 0:1], ALU.mult
                    )
                    nc.vector.tensor_tensor(vps[:, :], mv[:, 1:2], msq[:, :], ALU.add)
                    nc.vector.tensor_scalar_mul(
                        bnst[:, 2 * o : 2 * o + 1], mv[:, 0:1], float(HW)
                    )
                    nc.vector.tensor_scalar_mul(
                        bnst[:, 2 * o + 1 : 2 * o + 2], vps[:, :], float(HW)
                    )
                arin = dramp.tile([D, 2], F32, tag=f"arin{tag}", name=f"arin{tag}")
                arout = dramp.tile([D, 2], F32, tag=f"arout{tag}", name=f"arout{tag}")
                nc.sync.dma_start(arin[0:128, :], bnst[:, 0:2])
                nc.sync.dma_start(arin[128:256, :], bnst[:, 2:4])
                nc.gpsimd.collective_compute(
                    "AllReduce", ALU.add, replica_groups=groups,
                    ins=[arin[:, :].opt()], outs=[arout[:, :].opt()],
                )
                gst = small.tile([128, 4], F32, tag=f"gst{tag}", name=f"gst{tag}")
                nc.sync.dma_start(gst[:, 0:2], arout[0:128, :])
                nc.scalar.dma_start(gst[:, 2:4], arout[128:256, :])
                return gst

            def bn_coeffs(gst, gg, bb, tag):
                nm = small.tile([128, 2], F32, tag=f"nm{tag}", name=f"nm{tag}")
                ex2 = small.tile([128, 2], F32, tag=f"ex2{tag}", name=f"ex2{tag}")
                var = small.tile([128, 2], F32, tag=f"var{tag}", name=f"var{tag}")
                sd = small.tile([128, 2], F32, tag=f"sd{tag}", name=f"sd{tag}")
                rsd = small.tile([128, 2], F32, tag=f"rsd{tag}", name=f"rsd{tag}")
                scl = small.tile([128, 2], F32, tag=f"scl{tag}", name=f"scl{tag}")
                sh = small.tile([128, 2], F32, tag=f"sh{tag}", name=f"sh{tag}")
                gv = gst[:, :].rearrange("p (a b) -> p a b", a=2)
                nc.vector.tensor_scalar_mul(nm[:, :], gv[:, :, 0], -1.0 / BN_N)
                nc.vector.tensor_scalar_mul(ex2[:, :], gv[:, :, 1], 1.0 / BN_N)
                # var_neg = m^2 - E[x^2];  sd = sqrt(-var_neg + eps)
                nc.vector.tensor_tensor(var[:, :], nm[:, :], nm[:, :], ALU.mult)
                nc.vector.tensor_tensor(var[:, :], var[:, :], ex2[:, :], ALU.subtract)
                nc.scalar.activation(
                    sd[:, :], var[:, :], AF.Sqrt, bias=eps_t[:, 0:1], scale=-1.0,
                )
                nc.vector.reciprocal(rsd[:, :], sd[:, :])
                nc.vector.tensor_tensor(scl[:, :], rsd[:, :], gg[:, :], ALU.mult)
                nc.vector.tensor_tensor(sh[:, :], nm[:, :], scl[:, :], ALU.mult)
                nc.vector.tensor_tensor(sh[:, :], sh[:, :], bb[:, :], ALU.add)
                return scl, sh

            gst1 = bn_global(st1, "1")
            scl1, sh1 = bn_coeffs(gst1, g1, b1, "1")


            # ---------------- BN1 apply (chunked) + conv2 ----------------
            ip2 = [
                bigp.tile([128, 84, PW], BF16, tag="big", name=f"ip2_{c}")
                for c in range(2)
            ]
            for c in range(2):
                nc.gpsimd.memset(ip2[c][:, 0:2, :], 0.0)
                nc.gpsimd.memset(ip2[c][:, 82:84, :], 0.0)
                nc.gpsimd.memset(ip2[c][:, 2:82, 0:1], 0.0)
                nc.gpsimd.memset(ip2[c][:, 2:82, 81:82], 0.0)

            y2 = [
                b2p.tile([128, H, W], BF16, tag="b2", name=f"y2_{o}")
                for o in range(2)
            ]
            st2 = [
                small.tile([128, NRT, 6], F32, tag=f"st_{o}", name=f"st2_{o}")
                for o in range(2)
            ]
            ip2f = [_bd(ip2[c][:, :, :]) for c in range(2)]

            def conv2_tile(j):
                r0 = RT * j
                for o in range(2):
                    ps = psC.tile([128, NT], F32, tag="psC", name="psC")
                    idx = 0
                    for c in range(2):
                        for ky in range(3):
                            for kx in range(3):
                                s = (r0 + ky + 1) * PW + kx - 1
                                nc.tensor.matmul(
                                    ps[:, :],
                                    c2w[:, c * 9 + ky * 3 + kx,
                                        o * 128 : (o + 1) * 128],
                                    ip2f[c][:, s : s + NT],
                                    start=(idx == 0),
                                    stop=(idx == 17),
                                )
                                idx += 1
                    val = _r3(ps[:, :], RT)[:, :, 1:81]
                    nc.scalar.copy(y2[o][:, r0 : r0 + RT, :], val)
                    nc.vector.bn_stats(
                        st2[o][:, j, :],
                        _bd(y2[o][:, :, :])[:, r0 * W : r0 * W + 400],
                    )

            # apply chunk covering image rows [r0, r1), then its conv2 tiles
            apply_plan = [
                (0, 11, range(0, 2)),
                (11, 40, range(2, 7)),
                (40, 60, range(7, 11)),
                (60, 80, range(11, 16)),
            ]
            for pi, (r0, r1, tiles) in enumerate(apply_plan):
                for o in range(2):
                    nc.vector.tensor_scalar(
                        ip2[o][:, 2 + r0 : 2 + r1, 1:81],
                        y1[o][:, r0:r1, :],
                        scl1[:, o : o + 1],
                        sh1[:, o : o + 1],
                        ALU.mult,
                        ALU.add,
                    )
                for j in tiles:
                    conv2_tile(j)


            # ---------------- BN2 allreduce + residual + store ----------------
            gst2 = bn_global(st2, "2")
            scl2, sh2 = bn_coeffs(gst2, g2, b2, "2")

            for o in range(2):
                for k in range(8):
                    i = o * 8 + k
                    fsl = slice(800 * k, 800 * (k + 1))
                    tmp = fin.tile([128, 800], BF16, tag="tmp", name="tmp")
                    if i % 3 == 2:
                        nc.gpsimd.tensor_scalar(
                            tmp[:, :],
                            _bd(y2[o][:, :, :])[:, 800 * k : 800 * (k + 1)],
                            scl2[:, o : o + 1],
                            sh2[:, o : o + 1],
                            ALU.mult,
                            ALU.add,
                        )
                    else:
                        nc.scalar.activation(
                            _r3(tmp[:, :], 10),
                            y2[o][:, 10 * k : 10 * (k + 1), :],
                            AF.Identity,
                            bias=sh2[:, o : o + 1],
                            scale=scl2[:, o : o + 1],
                        )
                    ost = finO.tile([128, 800], BF16, tag="ost", name="ost")
                    eng = nc.gpsimd if i % 3 == 1 else nc.vector
                    eng.tensor_tensor(
                        _r3(ost[:, :], 10),
                        _r3(tmp[:, :], 10),
                        ip1[o][:, 2 + 10 * k : 12 + 10 * k, 1:81],
                        ALU.add,
                    )
                    dq = nc.scalar if i % 2 else nc.sync
                    dq.dma_start(out_d[o * 128 : (o + 1) * 128, fsl], ost[:, :])

    nc.compile()
    return nc


def _mblk():
    mb = np.zeros((8, 256), np.float32)
    for h in range(8):
        mb[h, h * 32 : (h + 1) * 32] = 1.0
    return mb.astype(ml_dtypes.bfloat16)


def _prep_inputs(feat0, zone_mask, w_q, w_k, w_v, conv1_w, bn1_g, bn1_b,
                 conv2_w, bn2_g, bn2_b, num_inside):
    B = feat0.shape[0]
    pos = np.asarray(zone_mask[:, :, 0])
    order = np.argsort(~pos, axis=1, kind="stable")
    assert np.array_equal(
        order[:, :num_inside],
        np.broadcast_to(np.arange(num_inside), (B, num_inside)),
    ), "kernel assumes inside positions are the first num_inside rows"
    assert num_inside == NI

    bf = ml_dtypes.bfloat16
    f32 = np.float32

    def wt(w):  # [dout, din] -> [128, 2, dout]: [p, ki, o] = w[o, ki*128+p]
        return np.ascontiguousarray(
            w.T.reshape(2, 128, D).transpose(1, 0, 2)
        ).astype(bf)

    def cw(w, nchunk):  # [O, I, 3, 3] -> [128, nchunk*9, O], tap idx = c*9+ky*3+kx
        o_, i_, _, _ = w.shape
        r = w.transpose(1, 2, 3, 0).reshape(nchunk, 128, 9, o_)
        return np.ascontiguousarray(
            r.transpose(1, 0, 2, 3).reshape(128, nchunk * 9, o_)
        ).astype(bf)

    common = {
        "wqt": wt(np.asarray(w_q, f32)),
        "wkt": wt(np.asarray(w_k, f32)),
        "wvt": wt(np.asarray(w_v, f32)),
        "c1w": cw(np.asarray(conv1_w, f32), 4),
        "c2w": cw(np.asarray(conv2_w, f32), 2),
        "bn1g": np.asarray(bn1_g, f32).reshape(D, 1),
        "bn1b": np.asarray(bn1_b, f32).reshape(D, 1),
        "bn2g": np.asarray(bn2_g, f32).reshape(D, 1),
        "bn2b": np.asarray(bn2_b, f32).reshape(D, 1),
        "mblk": _mblk(),
    }
    in_maps = []
    for b in range(NCORES):
        m = dict(common)
        ftT = np.asarray(feat0[b], f32).T.astype(bf)          # [256, 6400]
        pad = np.zeros((128, 2, 84, PW), bf)
        for mi in range(2):
            pad[:, mi, 2:82, 1:81] = ftT[mi * 128 : (mi + 1) * 128].reshape(
                128, 80, 80
            )
        m["ftpad"] = np.ascontiguousarray(pad.reshape(128, 2, 84 * PW))
        m["ftin"] = np.ascontiguousarray(
            ftT[:, :NI].reshape(2, 128, NI).transpose(1, 0, 2)
        )
        in_maps.append(m)
    return in_maps


def kernel(feat0, zone_mask, w_q, w_k, w_v, conv1_w, bn1_g, bn1_b,
           conv2_w, bn2_g, bn2_b, H=80, W=80, B=8, D=256, num_inside=3200,
           **_ignored):
    global LAST_EXEC_NS, LAST_MEAN_EXEC_NS
    if "nc" not in _cache:
        _cache["nc"] = build_nc()
    nc = _cache["nc"]

    in_maps = _prep_inputs(feat0, zone_mask, w_q, w_k, w_v, conv1_w, bn1_g,
                           bn1_b, conv2_w, bn2_g, bn2_b, int(num_inside))
    trace = os.environ.get("KERNEL_TRACE", "0") == "1"
    res = run_bass_kernel_spmd(nc, in_maps, list(range(NCORES)), trace=trace)
    LAST_EXEC_NS = res.exec_time_ns
    LAST_MEAN_EXEC_NS = res.mean_exec_time_ns
    out = np.empty((NCORES, HW, 256), np.float32)
    for b in range(NCORES):
        out[b] = res.results[b]["out_t"].T.astype(np.float32)
    return out


# revision 27
# speedup vs baseline: 1.0543x; 1.0543x over previous
"""Trainium2 Bass kernel for LoFTR-style encoder layer (sparse attention + convs).

Sharding: pure data-parallel over batch B=8 -> 8 NeuronCores (one batch
element per core). BN statistics are all-reduced across cores.

V2 restructure vs baseline:
  - host pre-pads + pre-casts feat0 to bf16 in the conv-input layout
    (ip1 f-halves DMA straight in; no on-device casts / big memsets)
  - conv1 f-only tiles (j=0..6) run while the K/V elu chain executes;
    tiles 7..15 interleave with the Q/message loop (attention step jj
    unlocks conv1 tile 7+jj)
  - DVE bn_stats/bn_aggr for BN statistics; conv psum evacuation on the
    scalar (ACT) engine; elu min/eps-add on gpsimd
  - BN1-apply chunked and interleaved with conv2 tiles
  - residual read from the resident bf16 ip1 copy (no f32 reload)
"""

import os
import sys

import numpy as np

for _p in ("/opt/trn_rl_repo", os.path.expanduser("~/.axon_site/_ro/trn_rl_repo")):
    if os.path.isdir(_p) and _p not in sys.path:
        sys.path.insert(0, _p)

import ml_dtypes

import concourse.bass as bass
import concourse.mybir as mybir
import concourse.tile as tile
from concourse import bacc
from concourse.bass_utils import run_bass_kernel_spmd

F32 = mybir.dt.float32
BF16 = mybir.dt.bfloat16
AF = mybir.ActivationFunctionType
ALU = mybir.AluOpType

NCORES = 8
H = W = 80
HW = H * W          # 6400
D = 256
NI = 3200           # inside positions (image rows 0..39)
NHEAD = 8
PW = W + 2          # 82 padded width
ATTN_EPS = 1e-6
BN_EPS = 1e-5
BN_N = float(NCORES * HW)

# conv row-tiling: 5 output rows per psum tile -> N = 5*82 = 410 <= 512
RT = 5
NRT = H // RT       # 16
NT = RT * PW        # 410

LAST_EXEC_NS = None
LAST_MEAN_EXEC_NS = None

_cache = {}


def _bd(ap3):
    return ap3.rearrange("p a b -> p (a b)")


def _r3(ap2, a):
    return ap2.rearrange("p (a b) -> p a b", a=a)


def build_nc():
    nc = bacc.Bacc(
        "TRN2", target_bir_lowering=False, debug=False, num_devices=NCORES
    )

    ftpad_d = nc.dram_tensor("ftpad", [128, 2, 84 * PW], BF16, kind="ExternalInput")
    ftin_d = nc.dram_tensor("ftin", [128, 2, NI], BF16, kind="ExternalInput")
    wqt_d = nc.dram_tensor("wqt", [128, 2, D], BF16, kind="ExternalInput")
    wkt_d = nc.dram_tensor("wkt", [128, 2, D], BF16, kind="ExternalInput")
    wvt_d = nc.dram_tensor("wvt", [128, 2, D], BF16, kind="ExternalInput")
    c1w_d = nc.dram_tensor("c1w", [128, 36, D], BF16, kind="ExternalInput")
    c2w_d = nc.dram_tensor("c2w", [128, 18, D], BF16, kind="ExternalInput")
    bn1g_d = nc.dram_tensor("bn1g", [D, 1], F32, kind="ExternalInput")
    bn1b_d = nc.dram_tensor("bn1b", [D, 1], F32, kind="ExternalInput")
    bn2g_d = nc.dram_tensor("bn2g", [D, 1], F32, kind="ExternalInput")
    bn2b_d = nc.dram_tensor("bn2b", [D, 1], F32, kind="ExternalInput")
    mblk_d = nc.dram_tensor("mblk", [8, 256], BF16, kind="ExternalInput")
    out_d = nc.dram_tensor("out_t", [D, HW], BF16, kind="ExternalOutput")

    groups = [list(range(NCORES))]

    with tile.TileContext(nc) as tc:
        with (
            tc.tile_pool(name="pers", bufs=1) as pers,
            tc.tile_pool(name="bigp", bufs=2) as bigp,
            tc.tile_pool(name="b2p", bufs=4) as b2p,
            tc.tile_pool(name="qtp", bufs=4) as qtp,
            tc.tile_pool(name="scr", bufs=5) as scr,
            tc.tile_pool(name="small", bufs=1) as small,
            tc.tile_pool(name="fin", bufs=3) as fin,
            tc.tile_pool(name="finO", bufs=4) as finO,
            tc.tile_pool(name="psA", bufs=3, space="PSUM") as psA,
            tc.tile_pool(name="psS", bufs=1, space="PSUM") as psS,
            tc.tile_pool(name="psC", bufs=4, space="PSUM") as psC,
            tc.tile_pool(name="dram", bufs=1, space="DRAM") as dramp,
        ):
            # ---------------- weights / consts / feat loads ----------------
            wqt = pers.tile([128, 2, D], BF16, tag="wqt", name="wqt")
            wkt = pers.tile([128, 2, D], BF16, tag="wkt", name="wkt")
            wvt = pers.tile([128, 2, D], BF16, tag="wvt", name="wvt")
            c1w = pers.tile([128, 36, D], BF16, tag="c1w", name="c1w")
            c2w = pers.tile([128, 18, D], BF16, tag="c2w", name="c2w")
            ftin = pers.tile([128, 2, NI], BF16, tag="ftin", name="ftin")
            maskblk = pers.tile([8, 256], BF16, tag="maskblk", name="maskblk")
            # padded conv1 input [512ch -> 4 chunks][84 rows, 82 cols]
            # tile row r+2 == image row r; guard rows/cols around.
            ip1 = [
                pers.tile([128, 84, PW], BF16, tag=f"ip1_{c}", name=f"ip1_{c}")
                for c in range(4)
            ]
            # dummy collective: warms the CC path / absorbs start skew
            # while phase-A compute proceeds
            dums = small.tile([8, 1], F32, tag="dums", name="dums")
            nc.gpsimd.memset(dums[:, :], 0.0)
            dumi = dramp.tile([8, 1], F32, tag="dumi", name="dumi")
            dumo = dramp.tile([8, 1], F32, tag="dumo", name="dumo")
            nc.scalar.dma_start(dumi[:, :], dums[:, :])
            nc.gpsimd.collective_compute(
                "AllReduce", ALU.add, replica_groups=groups,
                ins=[dumi[:, :].opt()], outs=[dumo[:, :].opt()],
            )
            nc.scalar.dma_start(wkt[:, :, :], wkt_d[:, :, :])
            for fc in range(4):
                nc.sync.dma_start(
                    ftin[:, :, 800 * fc : 800 * (fc + 1)],
                    ftin_d[:, :, 800 * fc : 800 * (fc + 1)],
                )
            nc.scalar.dma_start(wvt[:, :, :], wvt_d[:, :, :])
            for m in range(2):
                nc.sync.dma_start(ip1[m][:, 0:42, :], _r3(ftpad_d[:, m, :], 84)[:, 0:42, :])
            nc.scalar.dma_start(c1w[:, 0:18, :], c1w_d[:, 0:18, :])
            for m in range(2):
                nc.sync.dma_start(ip1[m][:, 42:84, :], _r3(ftpad_d[:, m, :], 84)[:, 42:84, :])
            nc.sync.dma_start(c1w[:, 18:36, :], c1w_d[:, 18:36, :])
            nc.scalar.dma_start(wqt[:, :, :], wqt_d[:, :, :])
            nc.sync.dma_start(maskblk[:, :], mblk_d[:, :])
            nc.sync.dma_start(c2w[:, :, :], c2w_d[:, :, :])

            eps_t = small.tile([128, 1], F32, tag="eps_t", name="eps_t")
            nc.gpsimd.memset(eps_t[:, :], BN_EPS)
            epsA = small.tile([128, 1], F32, tag="epsA", name="epsA")
            nc.gpsimd.memset(epsA[:, :], ATTN_EPS)
            g1 = small.tile([128, 2], F32, tag="g1", name="g1")
            b1 = small.tile([128, 2], F32, tag="b1", name="b1")
            g2 = small.tile([128, 2], F32, tag="g2", name="g2")
            b2 = small.tile([128, 2], F32, tag="b2", name="b2")
            for o in range(2):
                sl = slice(o * 128, (o + 1) * 128)
                nc.sync.dma_start(g1[:, o : o + 1], bn1g_d[sl, :])
                nc.sync.dma_start(b1[:, o : o + 1], bn1b_d[sl, :])
                nc.sync.dma_start(g2[:, o : o + 1], bn2g_d[sl, :])
                nc.sync.dma_start(b2[:, o : o + 1], bn2b_d[sl, :])

            # t-halves: zero region (image rows < 40 and halo)
            for c in (2, 3):
                nc.gpsimd.memset(ip1[c][:, 0:42, :], 0.0)
                nc.gpsimd.memset(ip1[c][:, 82:84, :], 0.0)
                nc.gpsimd.memset(ip1[c][:, 42:82, 0:1], 0.0)
                nc.gpsimd.memset(ip1[c][:, 42:82, 81:82], 0.0)

            # ---------------- conv1 machinery ----------------
            # y1/y2 hold valid cols only ([128, 80, 80])
            y1 = [
                b2p.tile([128, H, W], BF16, tag="b2", name=f"y1_{o}")
                for o in range(2)
            ]
            st1 = [
                small.tile([128, NRT, 6], F32, tag=f"st_{o}", name=f"st1_{o}")
                for o in range(2)
            ]
            ip1f = [_bd(ip1[c][:, :, :]) for c in range(4)]

            def conv1_half(j, o):
                r0 = RT * j
                taps = []
                for c in range(4):
                    for ky in range(3):
                        if c >= 2 and r0 + ky + 4 < 41:
                            continue  # t-channel rows all zero
                        for kx in range(3):
                            taps.append((c, ky, kx))
                ps = psC.tile([128, NT], F32, tag="psC", name="psC")
                for idx, (c, ky, kx) in enumerate(taps):
                    s = (r0 + ky + 1) * PW + kx - 1
                    nc.tensor.matmul(
                        ps[:, :],
                        c1w[:, c * 9 + ky * 3 + kx, o * 128 : (o + 1) * 128],
                        ip1f[c][:, s : s + NT],
                        start=(idx == 0),
                        stop=(idx == len(taps) - 1),
                    )
                val = _r3(ps[:, :], RT)[:, :, 1:81]
                nc.scalar.copy(y1[o][:, r0 : r0 + RT, :], val)
                nc.vector.bn_stats(
                    st1[o][:, j, :],
                    _bd(y1[o][:, :, :])[:, r0 * W : r0 * W + 400],
                )

            def conv1_tile(j):
                conv1_half(j, 0)
                conv1_half(j, 1)

            # ---------------- K / V projections ([s, c] layout) ----------------
            ke = bigp.tile([128, 25, D], BF16, tag="big", name="ke")
            ve = bigp.tile([128, 25, D + 1], BF16, tag="big", name="ve")
            nc.gpsimd.memset(ve[:, :, :], 1.0)  # col 0 of each chunk = ones
            for i in range(25):
                ps = psA.tile([128, D], F32, tag="psA", name="psA")
                for ki in range(2):
                    nc.tensor.matmul(
                        ps[:, :],
                        ftin[:, ki, i * 128 : (i + 1) * 128],
                        wkt[:, ki, :],
                        start=(ki == 0),
                        stop=(ki == 1),
                    )
                # elu(x)+1 = relu(x) + exp(min(x,0))
                sm = scr.tile([128, 400], F32, tag="scr", name="sm")
                se = scr.tile([128, 400], F32, tag="scr", name="se")
                nc.vector.tensor_scalar_min(sm[:, :D], ps[:, :], 0.0)
                nc.scalar.activation(se[:, :D], sm[:, :D], AF.Exp)
                nc.vector.scalar_tensor_tensor(
                    ke[:, i, :], ps[:, :], 0.0, se[:, :D], ALU.max, ALU.add
                )

                ps2 = psC.tile([128, D], F32, tag="psC", name="psC")
                for ki in range(2):
                    nc.tensor.matmul(
                        ps2[:, :],
                        ftin[:, ki, i * 128 : (i + 1) * 128],
                        wvt[:, ki, :],
                        start=(ki == 0),
                        stop=(ki == 1),
                    )
                nc.scalar.copy(ve[:, i, 1:], ps2[:, :])

                if i % 4 == 3:
                    conv1_tile(i // 4)

            # ---------------- KV + Ksum -> block-diag BD ----------------
            bd = [
                pers.tile([128, 264], BF16, tag=f"bd{m}", name=f"bd{m}")
                for m in range(2)
            ]
            for m in range(2):
                psm = psA.tile([128, D + 1], F32, tag="psA", name="psA")
                for i in range(25):
                    nc.tensor.matmul(
                        psm[:, :],
                        ke[:, i, m * 128 : (m + 1) * 128],
                        ve[:, i, :],
                        start=(i == 0),
                        stop=(i == 24),
                    )
                nc.gpsimd.memset(bd[m][:, :], 0.0)
                for hh in range(4):
                    h = m * 4 + hh
                    lh = hh * 32
                    nc.vector.tensor_copy(
                        bd[m][lh : lh + 32, h * 32 : (h + 1) * 32],
                        psm[lh : lh + 32, 1 + h * 32 : 1 + (h + 1) * 32],
                    )
                    nc.vector.tensor_copy(
                        bd[m][lh : lh + 32, 256 + h : 257 + h],
                        psm[lh : lh + 32, 0:1],
                    )

            # ---------- per l-tile: Q^T proj + elu, S, message, scatter ----------
            # conv1 tile 7+jj interleaves after attention step jj
            for jj in range(8):
                qt = [
                    qtp.tile([128, 400], BF16, tag="qteT", name=f"qt{m}")
                    for m in range(2)
                ]
                for m in range(2):
                    ps = psA.tile([128, 400], F32, tag="psA", name="psA")
                    for ki in range(2):
                        nc.tensor.matmul(
                            ps[:, :],
                            wqt[:, ki, m * 128 : (m + 1) * 128],
                            ip1[ki][:, 42 + 5 * jj : 47 + 5 * jj, 1:81],
                            start=(ki == 0),
                            stop=(ki == 1),
                        )
                    sm = scr.tile([128, 400], F32, tag="scr", name="smq")
                    se = scr.tile([128, 400], F32, tag="scr", name="seq")
                    nc.vector.tensor_scalar_min(sm[:, :], ps[:, :], 0.0)
                    nc.scalar.activation(se[:, :], sm[:, :], AF.Exp)
                    nc.vector.scalar_tensor_tensor(
                        qt[m][:, :], ps[:, :], 0.0, se[:, :], ALU.max, ALU.add
                    )

                conv1_half(6 + jj, 0)

                pss = psS.tile([8, 400], F32, tag="psS", name="psS")
                for ki in range(2):
                    nc.tensor.matmul(
                        pss[:, :],
                        bd[ki][:, 256:264],
                        qt[ki][:, :],
                        start=(ki == 0),
                        stop=(ki == 1),
                    )
                sadd = scr.tile([128, 400], F32, tag="scr", name="sadd")
                rs = scr.tile([128, 400], BF16, tag="scr", name="rs")
                nc.vector.tensor_scalar_add(sadd[:8, :], pss[:, :], ATTN_EPS)
                with nc.allow_low_precision(reason="Z recip feeds bf16 conv"):
                    nc.vector.reciprocal(rs[:8, :], sadd[:8, :])

                conv1_half(6 + jj, 1)

                pre = psC.tile([128, 400], F32, tag="psC", name="psC")
                nc.tensor.matmul(pre[:, :], maskblk[:, 0:128], rs[:8, :])
                preb = scr.tile([128, 400], F32, tag="scr", name="preb")
                nc.scalar.copy(preb[:, :], pre[:, :])
                pre2 = psC.tile([128, 400], F32, tag="psC", name="psC")
                nc.tensor.matmul(pre2[:, :], maskblk[:, 128:256], rs[:8, :])
                preb2 = scr.tile([128, 400], F32, tag="scr", name="preb2")
                nc.scalar.copy(preb2[:, :], pre2[:, :])
                prebs = [preb, preb2]

                for m in range(2):
                    psg = psA.tile([128, 400], F32, tag="psA", name="psA")
                    for ki in range(2):
                        nc.tensor.matmul(
                            psg[:, :],
                            bd[ki][:, m * 128 : (m + 1) * 128],
                            qt[ki][:, :],
                            start=(ki == 0),
                            stop=(ki == 1),
                        )
                    psgb = scr.tile([128, 400], F32, tag="scr", name="psgb")
                    nc.scalar.copy(psgb[:, :], psg[:, :])
                    # l-tile jj = image rows 40+5jj..44+5jj -> tile rows 42+5jj..
                    nc.gpsimd.tensor_tensor(
                        ip1[2 + m][:, 42 + 5 * jj : 47 + 5 * jj, 1:81],
                        _r3(psgb[:, :], RT),
                        _r3(prebs[m][:, :], RT),
                        ALU.mult,
                    )

            conv1_tile(14)
            conv1_tile(15)

            # ---------------- BN1 stats allreduce ----------------
            def bn_global(st, tag):
                bnst = small.tile([128, 4], F32, tag=f"bnst{tag}", name=f"bnst{tag}")
                for o in range(2):
                    mv = small.tile([128, 2], F32, tag=f"mv{tag}{o}", name=f"mv{tag}{o}")
                    nc.vector.bn_aggr(mv[:, :], st[o][:, :, :])
                    msq = small.tile([128, 1], F32, tag=f"msq{tag}{o}", name=f"msq{tag}{o}")
                    vps = small.tile([128, 1], F32, tag=f"vps{tag}{o}", name=f"vps{tag}{o}")
                    nc.vector.tensor_tensor(
                        msq[:, :], mv[:, 0:1], mv[:, 0:1], ALU.mult
                    )
                    nc.vector.tensor_tensor(vps[:, :], mv[:, 1:2], msq[:, :], ALU.add)
                    nc.vector.tensor_scalar_mul(
                        bnst[:, 2 * o : 2 * o + 1], mv[:, 0:1], float(HW)
                    )
                    nc.vector.tensor_scalar_mul(
                        bnst[:, 2 * o + 1 : 2 * o + 2], vps[:, :], float(HW)
                    )
                arin = dramp.tile([D, 2], F32, tag=f"arin{tag}", name=f"arin{tag}")
                arout = dramp.tile([D, 2], F32, tag=f"arout{tag}", name=f"arout{tag}")
                nc.sync.dma_start(arin[0:128, :], bnst[:, 0:2])
                nc.sync.dma_start(arin[128:256, :], bnst[:, 2:4])
                nc.gpsimd.collective_compute(
                    "AllReduce", ALU.add, replica_groups=groups,
                    ins=[arin[:, :].opt()], outs=[arout[:, :].opt()],
                )
                gst = small.tile([128, 4], F32, tag=f"gst{tag}", name=f"gst{tag}")
                nc.sync.dma_start(gst[:, 0:2], arout[0:128, :])
                nc.scalar.dma_start(gst[:, 2:4], arout[128:256, :])
                return gst

            def bn_coeffs(gst, gg, bb, tag):
                nm = small.tile([128, 2], F32, tag=f"nm{tag}", name=f"nm{tag}")
                ex2 = small.tile([128, 2], F32, tag=f"ex2{tag}", name=f"ex2{tag}")
                var = small.tile([128, 2], F32, tag=f"var{tag}", name=f"var{tag}")
                sd = small.tile([128, 2], F32, tag=f"sd{tag}", name=f"sd{tag}")
                rsd = small.tile([128, 2], F32, tag=f"rsd{tag}", name=f"rsd{tag}")
                scl = small.tile([128, 2], F32, tag=f"scl{tag}", name=f"scl{tag}")
                sh = small.tile([128, 2], F32, tag=f"sh{tag}", name=f"sh{tag}")
                gv = gst[:, :].rearrange("p (a b) -> p a b", a=2)
                nc.vector.tensor_scalar_mul(nm[:, :], gv[:, :, 0], -1.0 / BN_N)
                nc.vector.tensor_scalar_mul(ex2[:, :], gv[:, :, 1], 1.0 / BN_N)
                # var_neg = m^2 - E[x^2];  sd = sqrt(-var_neg + eps)
                nc.vector.tensor_tensor(var[:, :], nm[:, :], nm[:, :], ALU.mult)
                nc.vector.tensor_tensor(var[:, :], var[:, :], ex2[:, :], ALU.subtract)
                nc.scalar.activation(
                    sd[:, :], var[:, :], AF.Sqrt, bias=eps_t[:, 0:1], scale=-1.0,
                )
                nc.vector.reciprocal(rsd[:, :], sd[:, :])
                nc.vector.tensor_tensor(scl[:, :], rsd[:, :], gg[:, :], ALU.mult)
                nc.vector.tensor_tensor(sh[:, :], nm[:, :], scl[:, :], ALU.mult)
                nc.vector.tensor_tensor(sh[:, :], sh[:, :], bb[:, :], ALU.add)
                return scl, sh

            gst1 = bn_global(st1, "1")
            scl1, sh1 = bn_coeffs(gst1, g1, b1, "1")


            # ---------------- BN1 apply (chunked) + conv2 ----------------
            ip2 = [
                bigp.tile([128, 84, PW], BF16, tag="big", name=f"ip2_{c}")
                for c in range(2)
            ]
            for c in range(2):
                nc.gpsimd.memset(ip2[c][:, 0:2, :], 0.0)
                nc.gpsimd.memset(ip2[c][:, 82:84, :], 0.0)
                nc.gpsimd.memset(ip2[c][:, 2:82, 0:1], 0.0)
                nc.gpsimd.memset(ip2[c][:, 2:82, 81:82], 0.0)

            y2 = [
                b2p.tile([128, H, W], BF16, tag="b2", name=f"y2_{o}")
                for o in range(2)
            ]
            st2 = [
                small.tile([128, NRT, 6], F32, tag=f"st_{o}", name=f"st2_{o}")
                for o in range(2)
            ]
            ip2f = [_bd(ip2[c][:, :, :]) for c in range(2)]

            def conv2_tile(j):
                r0 = RT * j
                for o in range(2):
                    ps = psC.tile([128, NT], F32, tag="psC", name="psC")
                    idx = 0
                    for c in range(2):
                        for ky in range(3):
                            for kx in range(3):
                                s = (r0 + ky + 1) * PW + kx - 1
                                nc.tensor.matmul(
                                    ps[:, :],
                                    c2w[:, c * 9 + ky * 3 + kx,
                                        o * 128 : (o + 1) * 128],
                                    ip2f[c][:, s : s + NT],
                                    start=(idx == 0),
                                    stop=(idx == 17),
                                )
                                idx += 1
                    val = _r3(ps[:, :], RT)[:, :, 1:81]
                    nc.scalar.copy(y2[o][:, r0 : r0 + RT, :], val)
                    nc.vector.bn_stats(
                        st2[o][:, j, :],
                        _bd(y2[o][:, :, :])[:, r0 * W : r0 * W + 400],
                    )

            # apply chunk covering image rows [r0, r1), then its conv2 tiles
            apply_plan = [
                (0, 11, range(0, 2)),
                (11, 40, range(2, 7)),
                (40, 60, range(7, 11)),
                (60, 80, range(11, 16)),
            ]
            for pi, (r0, r1, tiles) in enumerate(apply_plan):
                for o in range(2):
                    nc.vector.tensor_scalar(
                        ip2[o][:, 2 + r0 : 2 + r1, 1:81],
                        y1[o][:, r0:r1, :],
                        scl1[:, o : o + 1],
                        sh1[:, o : o + 1],
                        ALU.mult,
                        ALU.add,
                    )
                for j in tiles:
                    conv2_tile(j)


            # ---------------- BN2 allreduce + residual + store ----------------
            gst2 = bn_global(st2, "2")
            scl2, sh2 = bn_coeffs(gst2, g2, b2, "2")

            for o in range(2):
                for k in range(8):
                    i = o * 8 + k
                    fsl = slice(800 * k, 800 * (k + 1))
                    tmp = fin.tile([128, 800], BF16, tag="tmp", name="tmp")
                    if i % 3 == 2:
                        nc.gpsimd.tensor_scalar(
                            tmp[:, :],
                            _bd(y2[o][:, :, :])[:, 800 * k : 800 * (k + 1)],
                            scl2[:, o : o + 1],
                            sh2[:, o : o + 1],
                            ALU.mult,
                            ALU.add,
                        )
                    else:
                        nc.scalar.activation(
                            _r3(tmp[:, :], 10),
                            y2[o][:, 10 * k : 10 * (k + 1), :],
                            AF.Identity,
                            bias=sh2[:, o : o + 1],
                            scale=scl2[:, o : o + 1],
                        )
                    ost = finO.tile([128, 800], BF16, tag="ost", name="ost")
                    eng = nc.gpsimd if i % 3 == 1 else nc.vector
                    eng.tensor_tensor(
                        _r3(ost[:, :], 10),
                        _r3(tmp[:, :], 10),
                        ip1[o][:, 2 + 10 * k : 12 + 10 * k, 1:81],
                        ALU.add,
                    )
                    dq = nc.scalar if i % 2 else nc.sync
                    dq.dma_start(out_d[o * 128 : (o + 1) * 128, fsl], ost[:, :])

    nc.compile()
    return nc


def _mblk():
    mb = np.zeros((8, 256), np.float32)
    for h in range(8):
        mb[h, h * 32 : (h + 1) * 32] = 1.0
    return mb.astype(ml_dtypes.bfloat16)


def _prep_inputs(feat0, zone_mask, w_q, w_k, w_v, conv1_w, bn1_g, bn1_b,
                 conv2_w, bn2_g, bn2_b, num_inside):
    B = feat0.shape[0]
    pos = np.asarray(zone_mask[:, :, 0])
    order = np.argsort(~pos, axis=1, kind="stable")
    assert np.array_equal(
        order[:, :num_inside],
        np.broadcast_to(np.arange(num_inside), (B, num_inside)),
    ), "kernel assumes inside positions are the first num_inside rows"
    assert num_inside == NI

    bf = ml_dtypes.bfloat16
    f32 = np.float32

    def wt(w):  # [dout, din] -> [128, 2, dout]: [p, ki, o] = w[o, ki*128+p]
        return np.ascontiguousarray(
            w.T.reshape(2, 128, D).transpose(1, 0, 2)
        ).astype(bf)

    def cw(w, nchunk):  # [O, I, 3, 3] -> [128, nchunk*9, O], tap idx = c*9+ky*3+kx
        o_, i_, _, _ = w.shape
        r = w.transpose(1, 2, 3, 0).reshape(nchunk, 128, 9, o_)
        return np.ascontiguousarray(
            r.transpose(1, 0, 2, 3).reshape(128, nchunk * 9, o_)
        ).astype(bf)

    common = {
        "wqt": wt(np.asarray(w_q, f32)),
        "wkt": wt(np.asarray(w_k, f32)),
        "wvt": wt(np.asarray(w_v, f32)),
        "c1w": cw(np.asarray(conv1_w, f32), 4),
        "c2w": cw(np.asarray(conv2_w, f32), 2),
        "bn1g": np.asarray(bn1_g, f32).reshape(D, 1),
        "bn1b": np.asarray(bn1_b, f32).reshape(D, 1),
        "bn2g": np.asarray(bn2_g, f32).reshape(D, 1),
        "bn2b": np.asarray(bn2_b, f32).reshape(D, 1),
        "mblk": _mblk(),
    }
    in_maps = []
    for b in range(NCORES):
        m = dict(common)
        ftT = np.asarray(feat0[b], f32).T.astype(bf)          # [256, 6400]
        pad = np.zeros((128, 2, 84, PW), bf)
        for mi in range(2):
            pad[:, mi, 2:82, 1:81] = ftT[mi * 128 : (mi + 1) * 128].reshape(
                128, 80, 80
            )
        m["ftpad"] = np.ascontiguousarray(pad.reshape(128, 2, 84 * PW))
        m["ftin"] = np.ascontiguousarray(
            ftT[:, :NI].reshape(2, 128, NI).transpose(1, 0, 2)
        )
        in_maps.append(m)
    return in_maps


def kernel(feat0, zone_mask, w_q, w_k, w_v, conv1_w, bn1_g, bn1_b,
           conv2_w, bn2_g, bn2_b, H=80, W=80, B=8, D=256, num_inside=3200,
           **_ignored):
    global LAST_EXEC_NS, LAST_MEAN_EXEC_NS
    if "nc" not in _cache:
        _cache["nc"] = build_nc()
    nc = _cache["nc"]

    in_maps = _prep_inputs(feat0, zone_mask, w_q, w_k, w_v, conv1_w, bn1_g,
                           bn1_b, conv2_w, bn2_g, bn2_b, int(num_inside))
    trace = os.environ.get("KERNEL_TRACE", "0") == "1"
    res = run_bass_kernel_spmd(nc, in_maps, list(range(NCORES)), trace=trace)
    LAST_EXEC_NS = res.exec_time_ns
    LAST_MEAN_EXEC_NS = res.mean_exec_time_ns
    out = np.empty((NCORES, HW, 256), np.float32)
    for b in range(NCORES):
        out[b] = res.results[b]["out_t"].T.astype(np.float32)
    return out
